# revision 31
# baseline (speedup 1.0000x reference)
"""Causal self-attention (B=4, S=4096, D=64, H=4) on 8 TRN2 NeuronCores.

Sharding: 16 (batch, head) pairs, 2 per core (core c -> batch c//2, heads
(2*(c%2), 2*(c%2)+1)). Each core runs fused attention for its 2 pairs; no
cross-core communication.

Per-core program (SPMD):
  - scores computed TRANSPOSED (S.T = K_blk @ Q.T, keys on partitions) so
    P@V needs no transpose; softmax denominator comes from a ones column
    appended to V. No max-subtraction (scores are O(13), fp32 exp safe).
  - QKV projection emits qT/kT with replica g at partition rows 32g+0..15
    (rows 32g+16..31 zero), via [65,128]-stationary matmuls whose weight
    columns carry the replicas. The replicas feed 4-way row-tiled score
    matmuls (tile_position=(32g,0), LDWEIGHTS needs 32-aligned bases) for
    ~4x concurrency at K=16.
  - softmax exp is SPLIT between the scalar engine (exact spline exp) and
    the vector engine (Schraudolph bit-trick: int16 bits = s*128/ln2 + B2
    converted straight out of PSUM, bitcast as bf16), greedily balanced
    with a static cost model. The +-3% trick error is tolerable: the
    denominator uses the same approximate values so softmax
    self-normalizes most of it away.
  - causal mask: the otherwise-idle GpSimd engine zeroes the strictly-
    masked triangle of each diagonal 128x128 block AFTER exp
    (affine_select fill=0.0, SBUF bf16); fully-masked query sub-columns
    are simply never read by PV (qoff trimming). No mask tensor, no
    score-side masking at all.
  - PV runs 4-way COLUMN-tiled (tile_position=(0,32j)): two pairs x two
    256-query halves stream concurrently through disjoint 32-col groups
    of the PE array, quartering the dominant PV wall time. K=128 PV
    matmuls also keep the HAM clock gate warm.
  - output: [2, 17, S] f32 = unnormalized O.T plus denominator row;
    divide on host.
"""

import numpy as np
import ml_dtypes

_B, _S, _D = 4, 4096, 64
_H, _Dh = 4, 16
_NC = 8
_SCALE = 1.0 / np.sqrt(_Dh)
_NQB = _S // 512  # 8 query super-blocks of 512
_CHUNK = 3  # key blocks per exp chunk (3 PSUM banks, 2 bufs in flight)

# Schraudolph exp in bf16-bit space: bits = round(s * _A2 + _B2)
_A2 = 128.0 / np.log(2.0)
_B2 = 128.0 * (127.0 - 0.0430359)

_cache = {}


def _build_nc():
    import concourse.tile as tile
    from concourse import bacc, mybir

    bf = mybir.dt.bfloat16
    i16 = mybir.dt.int16
    f32 = mybir.dt.float32
    Exp = mybir.ActivationFunctionType.Exp
    MULT = mybir.AluOpType.mult
    ADD = mybir.AluOpType.add

    nc = bacc.Bacc("TRN2", target_bir_lowering=False, debug=False, num_devices=_NC)
    xT_d = nc.dram_tensor("xT", [_D + 1, _S], bf, kind="ExternalInput").ap()
    wqk_d = nc.dram_tensor("wqk", [_D + 1, 512], bf, kind="ExternalInput").ap()
    wv_d = nc.dram_tensor("wv", [_D + 1, 32], bf, kind="ExternalInput").ap()
    out_d = nc.dram_tensor("out", [2, 17, _S], f32, kind="ExternalOutput").ap()

    # static ACT/DVE balance bookkeeping (model ns)
    tbusy = {"act": 0.0, "dve": 0.0}

    def _cost(eng, fd):
        # PSUM-source 1x ops + measured per-op dead time (bubble/drain)
        return (fd + 352.0) / 1.2 + 530.0 if eng == "act" else (fd + 120.0) / 0.96 + 480.0

    def pick(fd):
        eng = (
            "act"
            if tbusy["act"] + _cost("act", fd) <= tbusy["dve"] + _cost("dve", fd)
            else "dve"
        )
        tbusy[eng] += _cost(eng, fd)
        return eng

    with tile.TileContext(nc) as tc:
        with tc.tile_pool(name="singles", bufs=1) as singles:
            xT = singles.tile([_D + 1, _S], bf, tag="xT")
            wqk = singles.tile([_D + 1, 512], bf, tag="wqk")
            wv = singles.tile([_D + 1, 32], bf, tag="wv")
            scratch = singles.tile([128, 8], f32, tag="scratch")
            # qkT2[p]: cols 0..S-1 = qT, cols S.. = kT; rows 32g+0..15 =
            # replica g (rows 32g+16..31 zero)
            qkT2 = [
                singles.tile([128, 2 * _S], bf, tag=f"qkT{p}", name=f"qkT{p}")
                for p in range(2)
            ]
            # Vt: per key block b: [p0 v(16) | one | p1 v(16) | one]
            Vt = singles.tile([128, 34 * 32], bf, tag="Vt")
            # pt rings: unit qi uses PT[qi % 2], pair p at offset p*512*nkb
            PT = [
                singles.tile([128, 2 * 2048 * 7], bf, tag="PT0", name="PT0"),
                singles.tile([128, 2 * 2048 * 8], bf, tag="PT1", name="PT1"),
            ]

            # warm the ACT exp table load at t=0 (no input deps)
            nc.vector.memset(scratch[:], 0.0)
            nc.scalar.activation(out=scratch[:], in_=scratch[:], func=Exp)
            nc.vector.memset(Vt[:], 1.0)

            # input DMAs; wqk + xT chunk 3 first (qi=7 needs them first)
            nc.sync.dma_start(out=wqk[:], in_=wqk_d)
            for c in (3, 0, 1, 2):
                nc.sync.dma_start(
                    out=xT[:, 1024 * c : 1024 * (c + 1)],
                    in_=xT_d[:, 1024 * c : 1024 * (c + 1)],
                )
            nc.sync.dma_start(out=wv[:], in_=wv_d)

            # ---- attention (QK projection interleaved into the qi=7 unit) ----
            with (
                tc.tile_pool(name="ps_sc", bufs=1, space="PSUM") as ps_sc,
                tc.tile_pool(name="ps_o", bufs=1, space="PSUM") as ps_o,
                tc.tile_pool(name="stg", bufs=3) as stg,
            ):
                # psum budget (8 banks): scA [128,1536] x1 = 3, scB
                # [128,1024] x2 = 4, po [128,512] x1 = 1. Chunks cycle
                # A,B,B so each tag's reuse is 3 chunks apart -> 3-deep
                # score/exp pipeline (PE stays out of the exp critical
                # chain). po bufs=1 is free: PV bursts and stg copies of
                # consecutive units never overlap.

                def sc_tile(nblk, kind, name="ps"):
                    if kind == 0:
                        return ps_sc.tile([128, 1536], f32, tag="scA", name=name)
                    return ps_sc.tile([128, 1024], f32, tag="scB", bufs=2, name=name)
                # QK projection, emitted chunk-at-a-time interleaved with the
                # qi=7 score phase. psum banks borrowed from ps_o's rotation.
                proj_order = [7, 0, 1, 2, 3, 4, 5, 6]
                proj_state = [0]

                def pump_proj(kchunk_needed):
                    # ensure kT chunks 0..kchunk_needed (and qT chunk 7) done
                    while proj_state[0] < len(proj_order) and (
                        proj_state[0] < kchunk_needed + 2
                    ):
                        c = proj_order[proj_state[0]]
                        proj_state[0] += 1
                        csl = slice(512 * c, 512 * (c + 1))
                        for p in range(2):
                            pq = sc_tile(2, 1, name="pq")
                            for qk in range(2):
                                nc.tensor.matmul(
                                    pq[:, 512 * qk : 512 * (qk + 1)],
                                    wqk[
                                        :,
                                        256 * p + 128 * qk : 256 * p + 128 * (qk + 1),
                                    ],
                                    xT[:, csl],
                                    start=True,
                                    stop=True,
                                )
                            src = pq[:, 0:1024].rearrange("p (h c) -> p h c", h=2)
                            dst = qkT2[p].rearrange("p (h c) -> p h c", h=2)[
                                :, :, csl
                            ]
                            if pick(1024) == "act":
                                nc.scalar.copy(dst, src)
                            else:
                                nc.vector.tensor_copy(dst, src)
                # V projection: emitted inside the qi=7 score phase (PE has
                # slack); psum banks borrowed from ps_o's tag rotation.
                def emit_v_proj():
                    for half in range(2):
                        pv = ps_o.tile([128, 512], f32, tag="po", name="pv")
                        for s in range(16):
                            blk = 16 * half + s
                            nc.tensor.matmul(
                                pv[:, 32 * s : 32 * (s + 1)],
                                xT[:, 128 * blk : 128 * (blk + 1)],
                                wv[:],
                                start=True,
                                stop=True,
                            )
                        src = pv.rearrange("p (s pr c) -> p s pr c", s=16, pr=2, c=16)
                        dst = Vt.rearrange("p (s pr c) -> p s pr c", s=32, pr=2, c=17)[
                            :, 16 * half : 16 * (half + 1), :, 0:16
                        ]
                        nc.vector.tensor_copy(dst, src)
                        tbusy["dve"] += _cost("dve", 512)

                class Unit:
                    """One (qi, both pairs) superblock."""

                    def __init__(self, qi):
                        self.qi = qi
                        self.nkb = 4 * qi + 4
                        self.pt = PT[qi % 2]
                        self.po = None
                        self.done = 0

                    def pt_ap(self, p, c0, c1):
                        base = p * 512 * self.nkb
                        return self.pt[:, base + c0 : base + c1]

                    def start_pv(self):
                        self.po = ps_o.tile([128, 512], f32, tag="po", name="po")

                    def emit_pv_upto(self, k):
                        """PV quads for key blocks [done, min(k, nkb)).
                        Stream j: pair j//2, query half j%2; psum region
                        partitions 32j..32j+17, cols = relative query."""
                        k = min(k, self.nkb)
                        for b in range(self.done, k):
                            jd = b - 4 * self.qi
                            for j in range(4):
                                p, hi = j // 2, j % 2
                                q0, q1 = 256 * hi, 256 * hi + 256
                                if jd > 0:
                                    q0 = max(q0, 128 * jd)
                                if q0 >= q1:
                                    continue
                                last = self.nkb - 1 if hi else min(self.nkb - 1, 4 * self.qi + 1)
                                nc.tensor.matmul(
                                    self.po[32 * j : 32 * j + 17, q0:q1],
                                    Vt[:, 34 * b + 17 * p : 34 * b + 17 * p + 17],
                                    self.pt_ap(p, 512 * b + q0, 512 * b + q1),
                                    start=(b == 0),
                                    stop=(b == last),
                                    tile_position=(0, 32 * j),
                                )
                        self.done = max(self.done, k)

                    def finish_pv(self):
                        self.emit_pv_upto(self.nkb)
                        ost = stg.tile([113, 512], f32, tag="ost", name="ost")
                        if pick(512) == "act":
                            nc.scalar.copy(ost[:], self.po[0:113, :])
                        else:
                            nc.vector.tensor_copy(ost[:], self.po[0:113, :])
                        qs = (nc.sync, nc.gpsimd, nc.sync, nc.gpsimd)
                        for j in range(4):
                            p, hi = j // 2, j % 2
                            c0 = 512 * self.qi + 256 * hi
                            qs[j].dma_start(
                                out=out_d[p][:, c0 : c0 + 256],
                                in_=ost[32 * j : 32 * j + 17, 256 * hi : 256 * hi + 256],
                            )

                prev = None
                for ui, qi in enumerate(reversed(range(_NQB))):
                    unit = Unit(qi)
                    nkb = unit.nkb
                    # chunk sizes cycle 3,2,2 (tags A,B,B)
                    chunks = []
                    b0 = 0
                    kind = 0
                    while b0 < nkb:
                        n = min(3 if kind == 0 else 2, nkb - b0)
                        chunks.append((b0, n, kind))
                        b0 += n
                        kind = (kind + 1) % 3
                    nch = len(chunks) * 2
                    ci = 0
                    for p in range(2):
                        for b0, nblk, kind in chunks:
                            ci += 1
                            if ui == 0:
                                pump_proj((b0 + nblk - 1) // 4)
                            if prev is not None:
                                prev.emit_pv_upto((ci * prev.nkb) // nch)
                            fd = 512 * nblk
                            ps = sc_tile(nblk, 1 if kind else 0)
                            for t in range(nblk):
                                b = b0 + t
                                g = b % 4
                                # K=32 with 16 zero rows: same product, but
                                # 4 concurrent 32-row tiles read as a BUSY
                                # PE to the HAM clock gate (K=16 reads as
                                # idle -> permanent 1.2 GHz throttle)
                                nc.tensor.matmul(
                                    ps[:, 512 * t : 512 * (t + 1)],
                                    qkT2[p][
                                        32 * g : 32 * g + 32,
                                        _S + 128 * b : _S + 128 * (b + 1),
                                    ],
                                    qkT2[p][
                                        32 * g : 32 * g + 32,
                                        512 * qi : 512 * (qi + 1),
                                    ],
                                    start=True,
                                    stop=True,
                                    tile_position=(32 * g, 0),
                                )
                            if ui == 0 and p == 1 and b0 == 0:
                                emit_v_proj()
                            # exp: ACT exact or DVE Schraudolph. Skip the
                            # fully-masked prefix (first block jd >= 1).
                            jd1 = b0 - 4 * qi
                            skip0 = 128 * jd1 if 0 < jd1 < 4 else 0
                            dst = unit.pt_ap(p, 512 * b0 + skip0, 512 * (b0 + nblk))
                            if pick(fd - skip0) == "act":
                                nc.scalar.activation(
                                    out=dst, in_=ps[:, skip0:fd], func=Exp
                                )
                            else:
                                nc.vector.tensor_scalar(
                                    dst.bitcast(i16),
                                    ps[:, skip0:fd],
                                    _A2,
                                    _B2,
                                    MULT,
                                    ADD,
                                )
                            # zero the strictly-masked triangle of diagonal
                            # blocks (GpSimd; post-exp, SBUF bf16)
                            for t in range(nblk):
                                b = b0 + t
                                jd = b - 4 * qi
                                if 0 <= jd < 4:
                                    tri = unit.pt_ap(
                                        p,
                                        512 * b + 128 * jd,
                                        512 * b + 128 * jd + 128,
                                    )
                                    nc.gpsimd.affine_select(
                                        out=tri,
                                        in_=tri,
                                        pattern=[[1, 128]],
                                        compare_op=mybir.AluOpType.is_ge,
                                        fill=0.0,
                                        base=0,
                                        channel_multiplier=-1,
                                    )
                    if prev is not None:
                        prev.finish_pv()
                    unit.start_pv()
                    prev = unit
                prev.finish_pv()

    nc.compile()
    return nc


def _get_nc():
    if "nc" not in _cache:
        _cache["nc"] = _build_nc()
    return _cache["nc"]


def _prepare_in_maps(x, Wq, bq, Wk, bk, Wv, bv):
    bf = ml_dtypes.bfloat16
    x = np.asarray(x, np.float32)
    ones = np.ones((1, _S), np.float32)

    def aug(W, b, h, scale=1.0):
        blk = np.concatenate(
            [W[h * _Dh : (h + 1) * _Dh, :], b[h * _Dh : (h + 1) * _Dh, None]],
            axis=1,
        )
        return (blk * scale).T.astype(np.float32)  # [D+1, Dh]

    in_maps = []
    for c in range(_NC):
        b_idx = c // 2
        heads = (2 * (c % 2), 2 * (c % 2) + 1)
        xT = np.concatenate([x[b_idx].T, ones], axis=0)  # [65, 4096]
        wqk_cols = []
        wv_cols = []
        z16 = np.zeros((_D + 1, _Dh), np.float32)
        for h in heads:
            q = aug(Wq, bq, h, _SCALE)
            k = aug(Wk, bk, h)
            for blk in (q, k):
                for _ in range(4):
                    wqk_cols.append(blk)
                    wqk_cols.append(z16)
            wv_cols.append(aug(Wv, bv, h))
        in_maps.append(
            {
                "xT": xT.astype(bf),
                "wqk": np.concatenate(wqk_cols, axis=1).astype(bf),
                "wv": np.concatenate(wv_cols, axis=1).astype(bf),
            }
        )
    return in_maps


def _assemble(results):
    final = np.empty((_B, _S, _D), np.float32)
    for c in range(_NC):
        b_idx = c // 2
        for p in range(2):
            h = 2 * (c % 2) + p
            o = np.asarray(results[c]["out"], np.float32)  # [2, 17, S]
            final[b_idx, :, h * _Dh : (h + 1) * _Dh] = (o[p, :16] / o[p, 16:17]).T
    return final


def _run(in_maps, trace=False, trace_kwargs=None):
    from concourse.bass_utils import run_bass_kernel_spmd

    nc = _get_nc()
    return run_bass_kernel_spmd(
        nc, in_maps, list(range(_NC)), trace=trace, **(trace_kwargs or {})
    )


def kernel(x, Wq, bq, Wk, bk, Wv, bv):
    in_maps = _prepare_in_maps(x, Wq, bq, Wk, bk, Wv, bv)
    res = _run(in_maps)
    return _assemble(res.results)


# revision 32
# speedup vs baseline: 1.0140x; 1.0140x over previous
"""Causal self-attention (B=4, S=4096, D=64, H=4) on 8 TRN2 NeuronCores.

Sharding: 16 (batch, head) pairs, 2 per core (core c -> batch c//2, heads
(2*(c%2), 2*(c%2)+1)). Each core runs fused attention for its 2 pairs; no
cross-core communication.

Per-core program (SPMD):
  - scores computed TRANSPOSED (S.T = K_blk @ Q.T, keys on partitions) so
    P@V needs no transpose; softmax denominator comes from a ones column
    appended to V. No max-subtraction (scores are O(13), fp32 exp safe).
  - QKV projection emits qT/kT with replica g at partition rows 32g+0..15
    (rows 32g+16..31 zero), via [65,128]-stationary matmuls whose weight
    columns carry the replicas. The replicas feed 4-way row-tiled score
    matmuls (tile_position=(32g,0), LDWEIGHTS needs 32-aligned bases) for
    ~4x concurrency at K=16.
  - softmax exp is SPLIT between the scalar engine (exact spline exp) and
    the vector engine (Schraudolph bit-trick: int16 bits = s*128/ln2 + B2
    converted straight out of PSUM, bitcast as bf16), greedily balanced
    with a static cost model. The +-3% trick error is tolerable: the
    denominator uses the same approximate values so softmax
    self-normalizes most of it away.
  - causal mask: the otherwise-idle GpSimd engine zeroes the strictly-
    masked triangle of each diagonal 128x128 block AFTER exp
    (affine_select fill=0.0, SBUF bf16); fully-masked query sub-columns
    are simply never read by PV (qoff trimming). No mask tensor, no
    score-side masking at all.
  - PV runs 4-way COLUMN-tiled (tile_position=(0,32j)): two pairs x two
    256-query halves stream concurrently through disjoint 32-col groups
    of the PE array, quartering the dominant PV wall time. K=128 PV
    matmuls also keep the HAM clock gate warm.
  - output: [2, 17, S] f32 = unnormalized O.T plus denominator row;
    divide on host.
"""

import numpy as np
import ml_dtypes

_B, _S, _D = 4, 4096, 64
_H, _Dh = 4, 16
_NC = 8
_SCALE = 1.0 / np.sqrt(_Dh)
_NQB = _S // 512  # 8 query super-blocks of 512
_CHUNK = 3  # key blocks per exp chunk (3 PSUM banks, 2 bufs in flight)

# Schraudolph exp in bf16-bit space: bits = round(s * _A2 + _B2)
_A2 = 128.0 / np.log(2.0)
_B2 = 128.0 * (127.0 - 0.0430359)

_cache = {}


def _build_nc():
    import concourse.tile as tile
    from concourse import bacc, mybir

    bf = mybir.dt.bfloat16
    i16 = mybir.dt.int16
    f32 = mybir.dt.float32
    Exp = mybir.ActivationFunctionType.Exp
    MULT = mybir.AluOpType.mult
    ADD = mybir.AluOpType.add

    nc = bacc.Bacc("TRN2", target_bir_lowering=False, debug=False, num_devices=_NC)
    xT_d = nc.dram_tensor("xT", [_D + 1, _S], bf, kind="ExternalInput").ap()
    wqk_d = nc.dram_tensor("wqk", [_D + 1, 512], bf, kind="ExternalInput").ap()
    wv_d = nc.dram_tensor("wv", [_D + 1, 32], bf, kind="ExternalInput").ap()
    out_d = nc.dram_tensor("out", [2, 17, _S], f32, kind="ExternalOutput").ap()

    # static ACT/DVE balance bookkeeping (model ns)
    tbusy = {"act": 0.0, "dve": 0.0}

    def _cost(eng, fd):
        # PSUM-source 1x ops + measured per-op dead time (bubble/drain)
        return (fd + 352.0) / 1.2 + 530.0 if eng == "act" else (fd + 120.0) / 0.96 + 480.0

    def pick(fd):
        eng = (
            "act"
            if tbusy["act"] + _cost("act", fd) <= tbusy["dve"] + _cost("dve", fd)
            else "dve"
        )
        tbusy[eng] += _cost(eng, fd)
        return eng

    with tile.TileContext(nc) as tc:
        with tc.tile_pool(name="singles", bufs=1) as singles:
            xT = singles.tile([_D + 1, _S], bf, tag="xT")
            wqk = singles.tile([_D + 1, 512], bf, tag="wqk")
            wv = singles.tile([_D + 1, 32], bf, tag="wv")
            scratch = singles.tile([128, 8], f32, tag="scratch")
            # qkT2[p]: cols 0..S-1 = qT, cols S.. = kT; rows 32g+0..15 =
            # replica g (rows 32g+16..31 zero)
            qkT2 = [
                singles.tile([128, 2 * _S], bf, tag=f"qkT{p}", name=f"qkT{p}")
                for p in range(2)
            ]
            # Vt: per key block b: [p0 v(16) | one | p1 v(16) | one]
            Vt = singles.tile([128, 34 * 32], bf, tag="Vt")
            # pt rings: unit qi uses PT[qi % 2], pair p at offset p*512*nkb
            PT = [
                singles.tile([128, 2 * 2048 * 7], bf, tag="PT0", name="PT0"),
                singles.tile([128, 2 * 2048 * 8], bf, tag="PT1", name="PT1"),
            ]

            # warm the ACT exp table load at t=0 (no input deps)
            nc.vector.memset(scratch[:], 0.0)
            nc.scalar.activation(out=scratch[:], in_=scratch[:], func=Exp)
            nc.vector.memset(Vt[:], 1.0)

            # input DMAs; wqk + xT chunk 3 first (qi=7 needs them first)
            nc.sync.dma_start(out=wqk[:], in_=wqk_d)
            for c in (3, 0, 1, 2):
                nc.sync.dma_start(
                    out=xT[:, 1024 * c : 1024 * (c + 1)],
                    in_=xT_d[:, 1024 * c : 1024 * (c + 1)],
                )
            nc.sync.dma_start(out=wv[:], in_=wv_d)

            # ---- attention (QK projection interleaved into the qi=7 unit) ----
            with (
                tc.tile_pool(name="ps_sc", bufs=1, space="PSUM") as ps_sc,
                tc.tile_pool(name="ps_o", bufs=1, space="PSUM") as ps_o,
                tc.tile_pool(name="stg", bufs=3) as stg,
            ):
                # psum budget (8 banks): scA [128,1536] x1 = 3, scB
                # [128,1024] x2 = 4, po [128,512] x1 = 1. Chunks cycle
                # A,B,B so each tag's reuse is 3 chunks apart -> 3-deep
                # score/exp pipeline (PE stays out of the exp critical
                # chain). po bufs=1 is free: PV bursts and stg copies of
                # consecutive units never overlap.

                def sc_tile(nblk, kind, name="ps"):
                    if kind == 0:
                        return ps_sc.tile([128, 1536], f32, tag="scA", name=name)
                    return ps_sc.tile([128, 1024], f32, tag="scB", bufs=2, name=name)
                # QK projection, emitted chunk-at-a-time interleaved with the
                # qi=7 score phase. psum banks borrowed from ps_o's rotation.
                proj_order = [7, 0, 1, 2, 3, 4, 5, 6]
                proj_state = [0]

                def pump_proj(kchunk_needed):
                    # ensure kT chunks 0..kchunk_needed (and qT chunk 7) done
                    while proj_state[0] < len(proj_order) and (
                        proj_state[0] < kchunk_needed + 2
                    ):
                        c = proj_order[proj_state[0]]
                        proj_state[0] += 1
                        csl = slice(512 * c, 512 * (c + 1))
                        for p in range(2):
                            pq = sc_tile(2, 1, name="pq")
                            for qk in range(2):
                                nc.tensor.matmul(
                                    pq[:, 512 * qk : 512 * (qk + 1)],
                                    wqk[
                                        :,
                                        256 * p + 128 * qk : 256 * p + 128 * (qk + 1),
                                    ],
                                    xT[:, csl],
                                    start=True,
                                    stop=True,
                                )
                            src = pq[:, 0:1024].rearrange("p (h c) -> p h c", h=2)
                            dst = qkT2[p].rearrange("p (h c) -> p h c", h=2)[
                                :, :, csl
                            ]
                            if pick(1024) == "act":
                                nc.scalar.copy(dst, src)
                            else:
                                nc.vector.tensor_copy(dst, src)
                # V projection: emitted inside the qi=7 score phase (PE has
                # slack); psum banks borrowed from ps_o's tag rotation.
                def emit_v_proj():
                    for half in range(2):
                        pv = ps_o.tile([128, 512], f32, tag="po", name="pv")
                        for s in range(16):
                            blk = 16 * half + s
                            nc.tensor.matmul(
                                pv[:, 32 * s : 32 * (s + 1)],
                                xT[:, 128 * blk : 128 * (blk + 1)],
                                wv[:],
                                start=True,
                                stop=True,
                            )
                        src = pv.rearrange("p (s pr c) -> p s pr c", s=16, pr=2, c=16)
                        dst = Vt.rearrange("p (s pr c) -> p s pr c", s=32, pr=2, c=17)[
                            :, 16 * half : 16 * (half + 1), :, 0:16
                        ]
                        nc.vector.tensor_copy(dst, src)
                        tbusy["dve"] += _cost("dve", 512)

                class Unit:
                    """One (qi, both pairs) superblock."""

                    def __init__(self, qi):
                        self.qi = qi
                        self.nkb = 4 * qi + 4
                        self.pt = PT[qi % 2]
                        self.po = None
                        self.done = 0

                    def pt_ap(self, p, c0, c1):
                        base = p * 512 * self.nkb
                        return self.pt[:, base + c0 : base + c1]

                    def start_pv(self):
                        self.po = ps_o.tile([128, 512], f32, tag="po", name="po")

                    def emit_pv_upto(self, k):
                        """PV quads for key blocks [done, min(k, nkb)).
                        Stream j: pair j//2, query half j%2; psum region
                        partitions 32j..32j+17, cols = relative query."""
                        k = min(k, self.nkb)
                        for b in range(self.done, k):
                            jd = b - 4 * self.qi
                            for j in range(4):
                                p, hi = j // 2, j % 2
                                q0, q1 = 256 * hi, 256 * hi + 256
                                if jd > 0:
                                    q0 = max(q0, 128 * jd)
                                if q0 >= q1:
                                    continue
                                last = self.nkb - 1 if hi else min(self.nkb - 1, 4 * self.qi + 1)
                                nc.tensor.matmul(
                                    self.po[32 * j : 32 * j + 17, q0:q1],
                                    Vt[:, 34 * b + 17 * p : 34 * b + 17 * p + 17],
                                    self.pt_ap(p, 512 * b + q0, 512 * b + q1),
                                    start=(b == 0),
                                    stop=(b == last),
                                    tile_position=(0, 32 * j),
                                )
                        self.done = max(self.done, k)

                    def finish_pv(self):
                        self.emit_pv_upto(self.nkb)
                        ost = stg.tile([113, 512], f32, tag="ost", name="ost")
                        if pick(512) == "act":
                            nc.scalar.copy(ost[:], self.po[0:113, :])
                        else:
                            nc.vector.tensor_copy(ost[:], self.po[0:113, :])
                        qs = (nc.sync, nc.gpsimd, nc.sync, nc.gpsimd)
                        for j in range(4):
                            p, hi = j // 2, j % 2
                            c0 = 512 * self.qi + 256 * hi
                            qs[j].dma_start(
                                out=out_d[p][:, c0 : c0 + 256],
                                in_=ost[32 * j : 32 * j + 17, 256 * hi : 256 * hi + 256],
                            )

                prev = None
                for ui, qi in enumerate(reversed(range(_NQB))):
                    unit = Unit(qi)
                    nkb = unit.nkb
                    # chunk sizes cycle 3,2,2 (tags A,B,B)
                    chunks = []
                    b0 = 0
                    kind = 0
                    while b0 < nkb:
                        n = min(3 if kind == 0 else 2, nkb - b0)
                        chunks.append((b0, n, kind))
                        b0 += n
                        kind = (kind + 1) % 3
                    nch = len(chunks) * 2
                    ci = 0
                    for p in range(2):
                        for b0, nblk, kind in chunks:
                            ci += 1
                            if ui == 0:
                                pump_proj((b0 + nblk - 1) // 4)
                            if prev is not None and ci in (nch // 3, (2 * nch) // 3):
                                prev.emit_pv_upto(
                                    prev.nkb if ci >= (2 * nch) // 3 else prev.nkb // 2
                                )
                            fd = 512 * nblk
                            ps = sc_tile(nblk, 1 if kind else 0)
                            for t in range(nblk):
                                b = b0 + t
                                g = b % 4
                                # K=32 with 16 zero rows: same product, but
                                # 4 concurrent 32-row tiles read as a BUSY
                                # PE to the HAM clock gate (K=16 reads as
                                # idle -> permanent 1.2 GHz throttle)
                                nc.tensor.matmul(
                                    ps[:, 512 * t : 512 * (t + 1)],
                                    qkT2[p][
                                        32 * g : 32 * g + 32,
                                        _S + 128 * b : _S + 128 * (b + 1),
                                    ],
                                    qkT2[p][
                                        32 * g : 32 * g + 32,
                                        512 * qi : 512 * (qi + 1),
                                    ],
                                    start=True,
                                    stop=True,
                                    tile_position=(32 * g, 0),
                                )
                            if ui == 0 and p == 1 and b0 == 0:
                                emit_v_proj()
                            # exp: ACT exact or DVE Schraudolph. Skip the
                            # fully-masked prefix (first block jd >= 1).
                            jd1 = b0 - 4 * qi
                            skip0 = 128 * jd1 if 0 < jd1 < 4 else 0
                            dst = unit.pt_ap(p, 512 * b0 + skip0, 512 * (b0 + nblk))
                            if pick(fd - skip0) == "act":
                                nc.scalar.activation(
                                    out=dst, in_=ps[:, skip0:fd], func=Exp
                                )
                            else:
                                nc.vector.tensor_scalar(
                                    dst.bitcast(i16),
                                    ps[:, skip0:fd],
                                    _A2,
                                    _B2,
                                    MULT,
                                    ADD,
                                )
                            # zero the strictly-masked triangle of diagonal
                            # blocks (GpSimd; post-exp, SBUF bf16)
                            for t in range(nblk):
                                b = b0 + t
                                jd = b - 4 * qi
                                if 0 <= jd < 4:
                                    tri = unit.pt_ap(
                                        p,
                                        512 * b + 128 * jd,
                                        512 * b + 128 * jd + 128,
                                    )
                                    nc.gpsimd.affine_select(
                                        out=tri,
                                        in_=tri,
                                        pattern=[[1, 128]],
                                        compare_op=mybir.AluOpType.is_ge,
                                        fill=0.0,
                                        base=0,
                                        channel_multiplier=-1,
                                    )
                    if prev is not None:
                        prev.finish_pv()
                    unit.start_pv()
                    prev = unit
                prev.finish_pv()

    nc.compile()
    return nc


def _get_nc():
    if "nc" not in _cache:
        _cache["nc"] = _build_nc()
    return _cache["nc"]


def _prepare_in_maps(x, Wq, bq, Wk, bk, Wv, bv):
    bf = ml_dtypes.bfloat16
    x = np.asarray(x, np.float32)
    ones = np.ones((1, _S), np.float32)

    def aug(W, b, h, scale=1.0):
        blk = np.concatenate(
            [W[h * _Dh : (h + 1) * _Dh, :], b[h * _Dh : (h + 1) * _Dh, None]],
            axis=1,
        )
        return (blk * scale).T.astype(np.float32)  # [D+1, Dh]

    in_maps = []
    for c in range(_NC):
        b_idx = c // 2
        heads = (2 * (c % 2), 2 * (c % 2) + 1)
        xT = np.concatenate([x[b_idx].T, ones], axis=0)  # [65, 4096]
        wqk_cols = []
        wv_cols = []
        z16 = np.zeros((_D + 1, _Dh), np.float32)
        for h in heads:
            q = aug(Wq, bq, h, _SCALE)
            k = aug(Wk, bk, h)
            for blk in (q, k):
                for _ in range(4):
                    wqk_cols.append(blk)
                    wqk_cols.append(z16)
            wv_cols.append(aug(Wv, bv, h))
        in_maps.append(
            {
                "xT": xT.astype(bf),
                "wqk": np.concatenate(wqk_cols, axis=1).astype(bf),
                "wv": np.concatenate(wv_cols, axis=1).astype(bf),
            }
        )
    return in_maps


def _assemble(results):
    final = np.empty((_B, _S, _D), np.float32)
    for c in range(_NC):
        b_idx = c // 2
        for p in range(2):
            h = 2 * (c % 2) + p
            o = np.asarray(results[c]["out"], np.float32)  # [2, 17, S]
            final[b_idx, :, h * _Dh : (h + 1) * _Dh] = (o[p, :16] / o[p, 16:17]).T
    return final


def _run(in_maps, trace=False, trace_kwargs=None):
    from concourse.bass_utils import run_bass_kernel_spmd

    nc = _get_nc()
    return run_bass_kernel_spmd(
        nc, in_maps, list(range(_NC)), trace=trace, **(trace_kwargs or {})
    )


def kernel(x, Wq, bq, Wk, bk, Wv, bv):
    in_maps = _prepare_in_maps(x, Wq, bq, Wk, bk, Wv, bv)
    res = _run(in_maps)
    return _assemble(res.results)
